# revision 1
# baseline (speedup 1.0000x reference)
"""KNN retrieval kernel for Trainium2 (8 NeuronCores, data-parallel over queries).

Problem: for each query row x[i] (N=16384, DIM=16), find j* = argmin_j ||xb[j]-x[i]||
over M=16384 reference rows and return y[j*].

Device algorithm (per core, 2048 queries = 16 row-blocks of 128):
  ms[i,j] = 2<x_i, xb_j> - ||xb_j||^2   (argmax_j ms == argmin_j dist)
  - PE: per row-block, 8 PSUM chunks of 2048 j; each chunk = 4 matmuls of 512
    with a K=50 split-bf16 contraction (hi*hi + b2 + lo*hi + hi*lo), 2-way
    row-packed into PE quadrants. The two per-row-block weight groups stay
    resident across all 8 chunks, so only the first chunk's matmuls load
    weights (ldweights=False on the rest).
  - DVE: tensor_reduce max over 32-wide subblocks straight from PSUM
    -> SM[128, 512] per row-block (fp32, near-exact scores).
  - top-1 subblock: DVE prefix-max scan over SM, then ScalarE Sign
    (bias=rowmax, scale=-1) with accumulate: count of prefix<max = first
    argmax subblock index.
  - recheck: per row-block, GPSIMD indirect-gathers the 32-wide window
    (17 fp32 per j) from DRAM; DVE re-dots exactly in fp32 (batched per 4
    row-blocks); within-window argmax again via scan + ScalarE count.
  - j* = 32*subblock + offset; GPSIMD indirect-gathers y[j*].
Host: builds split-bf16 packed operands, the window table, shards queries
8 ways, reassembles.
"""

import sys

sys.path.insert(0, "/opt/trn_rl_repo")

import numpy as np

N, M, DIM = 16384, 16384, 16
NCORES = 8
NQ = N // NCORES
RB = 128
JT = 512
CHUNK = 2048
TPG = 4
K_SPL = 50
WSUB = 32
NSUB = M // WSUB          # 512
K_AUG = 17
WK = WSUB * K_AUG         # 544
GRP = 4

REUSE_WEIGHTS = True


def build_nc_v2(nq=NQ, m=M, reuse_weights=REUSE_WEIGHTS):
    from contextlib import ExitStack

    import concourse.bacc as bacc
    import concourse.bass as bass
    import concourse.mybir as mybir
    import concourse.tile as tile
    from concourse.bass import IndirectOffsetOnAxis

    fp32 = mybir.dt.float32
    bf16 = mybir.dt.bfloat16
    u32 = mybir.dt.uint32

    n_rb = nq // RB
    n_chunk = m // CHUNK
    NEGINF = float(np.float32(-3.0e38))

    nc = bacc.Bacc("TRN2", target_bir_lowering=False, debug=False)

    xq_d = nc.dram_tensor("xq4", [128, nq], bf16, kind="ExternalInput")
    xb_d = nc.dram_tensor("xbp", [128, n_chunk * 2 * JT], bf16,
                          kind="ExternalInput")
    xw_d = nc.dram_tensor("xw", [NSUB, WK], fp32, kind="ExternalInput")
    xqr_d = nc.dram_tensor("xqr", [128, n_rb * K_AUG], fp32,
                           kind="ExternalInput")
    y_d = nc.dram_tensor("ytab", [m, 1], fp32, kind="ExternalInput")
    out_d = nc.dram_tensor("yout", [128, n_rb], fp32, kind="ExternalOutput")

    with tile.TileContext(nc) as tc:
        with ExitStack() as ctx:
            consts = ctx.enter_context(tc.tile_pool(name="consts", bufs=1))
            psum_pool = ctx.enter_context(
                tc.tile_pool(name="ps", bufs=2, space=bass.MemorySpace.PSUM))
            smpool = ctx.enter_context(tc.tile_pool(name="sm", bufs=3))
            wpool = ctx.enter_context(tc.tile_pool(name="w", bufs=3))
            gpool = ctx.enter_context(tc.tile_pool(name="g", bufs=4))

            xq4 = consts.tile([128, nq], bf16)
            xb = consts.tile([128, n_chunk * 2 * JT], bf16)
            xqr = consts.tile([128, n_rb * K_AUG], fp32)
            # first matmul's operands first: xb chunk 0, then row-block 0's
            # queries, then the bulk loads
            nc.sync.dma_start(xb[:, 0:2 * JT], xb_d[:, 0:2 * JT])
            nc.sync.dma_start(xq4[:, 0:RB], xq_d[:, 0:RB])
            nc.sync.dma_start(xq4[:, RB:], xq_d[:, RB:])
            for t in range(1, n_chunk):
                nc.sync.dma_start(xb[:, t * 2 * JT:(t + 1) * 2 * JT],
                                  xb_d[:, t * 2 * JT:(t + 1) * 2 * JT])
            nc.sync.dma_start(xqr[:], xqr_d[:])

            group_starts = [0, 4, 8, 12, 14, 15]

            Sf = consts.tile([128, n_rb], fp32)    # top subblock id (count)
            Cf = consts.tile([128, n_rb], fp32)    # within-window argmax
            IDXu = consts.tile([128, n_rb], u32)
            Dd = consts.tile([128, n_rb * WSUB], fp32)
            tC = consts.tile([128, n_rb], fp32)
            JI = consts.tile([128, n_rb], u32)
            Yg = consts.tile([128, n_rb], fp32)

            def emit_mms(rb, t, ps):
                # K=50 split-bf16 contraction, 2-way row packing; weight
                # groups for s=0/1 live in different PE quadrants, so after
                # the first chunk of a row-block they stay resident.
                for v in range(2):
                    for s in range(2):
                        u = 2 * v + s
                        mm = nc.tensor.matmul(
                            ps[:, u * JT:(u + 1) * JT],
                            xq4[64 * s:64 * s + K_SPL,
                                rb * RB:(rb + 1) * RB],
                            xb[64 * s:64 * s + K_SPL,
                               (t * 2 + v) * JT:(t * 2 + v + 1) * JT],
                            start=True,
                            stop=True,
                            tile_position=(64 * s, 0),
                        )
                        if reuse_weights and t > 0:
                            mm.ins.ldweights = False

            def emit_group_head(g, r0, grp):
                """GPSIMD side of the recheck (gathers + multiply), started
                at the group boundary so it runs under the next row-block's
                drain. Returns the Wt tile for the deferred DVE side."""
                nc.vector.tensor_copy(IDXu[:, r0:r0 + grp],
                                      Sf[:, r0:r0 + grp])
                Wt = wpool.tile([128, grp * WK], fp32, name=f"wt{g}",
                                tag="wt")
                for ri in range(grp):
                    rb = r0 + ri
                    nc.gpsimd.indirect_dma_start(
                        Wt[:, ri * WK:(ri + 1) * WK], None, xw_d[:],
                        IndirectOffsetOnAxis(ap=IDXu[:, rb:rb + 1], axis=0))
                wv = Wt[:].rearrange("p (r c k) -> p r c k", r=grp, k=K_AUG)
                xq_b = (xqr[:, r0 * K_AUG:(r0 + grp) * K_AUG]
                        .rearrange("p (r b k) -> p r b k", r=grp, b=1)
                        .to_broadcast([128, grp, WSUB, K_AUG]))
                nc.gpsimd.tensor_tensor(wv, wv, xq_b,
                                        op=mybir.AluOpType.mult)
                return Wt

            def emit_group_tail(g, r0, grp, Wt):
                """DVE side of the recheck; emitted a few chunks into the
                next row-block so the in-order DVE queue never blocks on the
                GPSIMD multiply."""
                nc.vector.tensor_reduce(
                    Dd[:, r0 * WSUB:(r0 + grp) * WSUB],
                    Wt[:].rearrange("p (c k) -> p c k", k=K_AUG),
                    mybir.AxisListType.X, mybir.AluOpType.add)
                # within-window argmax via scan + count, per rb
                for ri in range(grp):
                    rb = r0 + ri
                    pmc = gpool.tile([128, WSUB], fp32, name=f"pmc{rb}",
                                     tag="pmc")
                    gc = gpool.tile([128, 1], fp32, name=f"gc{rb}", tag="gc")
                    dslice = Dd[:, rb * WSUB:(rb + 1) * WSUB]
                    nc.vector.tensor_tensor_scan(
                        pmc[:], dslice, dslice, NEGINF,
                        mybir.AluOpType.max, mybir.AluOpType.bypass)
                    nc.vector.tensor_copy(gc[:], pmc[:, WSUB - 1:WSUB])
                    nc.scalar.activation(
                        pmc[:], pmc[:], mybir.ActivationFunctionType.Sign,
                        bias=gc[:], scale=-1.0,
                        accum_out=Cf[:, rb:rb + 1])
                # j* = s*32 + c and y[j*] gather for this group, so the
                # final group's tail is the only exposed epilogue work
                nc.vector.scalar_tensor_tensor(
                    tC[:, r0:r0 + grp], Sf[:, r0:r0 + grp], float(WSUB),
                    Cf[:, r0:r0 + grp],
                    mybir.AluOpType.mult, mybir.AluOpType.add)
                nc.vector.tensor_copy(JI[:, r0:r0 + grp],
                                      tC[:, r0:r0 + grp])
                for ri in range(grp):
                    rb = r0 + ri
                    nc.gpsimd.indirect_dma_start(
                        Yg[:, rb:rb + 1], None, y_d[:],
                        IndirectOffsetOnAxis(ap=JI[:, rb:rb + 1], axis=0))

            deferred = None
            for rb in range(n_rb):
                SM = smpool.tile([128, NSUB], fp32, name=f"sm{rb}", tag="sm")
                for t in range(n_chunk):
                    ps = psum_pool.tile([128, CHUNK], fp32)
                    emit_mms(rb, t, ps)
                    cps = CHUNK // WSUB
                    nc.vector.tensor_reduce(
                        SM[:, t * cps:(t + 1) * cps],
                        ps[:].rearrange("p (s w) -> p s w", w=WSUB),
                        mybir.AxisListType.X, mybir.AluOpType.max)
                    if t == 5 and deferred is not None:
                        emit_group_tail(*deferred)
                        deferred = None
                # top-1 subblock via scan + count
                pm = smpool.tile([128, NSUB], fp32, name=f"pm{rb}", tag="pm")
                gt = gpool.tile([128, 1], fp32, name=f"gt{rb}", tag="gt")
                nc.vector.tensor_tensor_scan(
                    pm[:], SM[:], SM[:], NEGINF,
                    mybir.AluOpType.max, mybir.AluOpType.bypass)
                nc.vector.tensor_copy(gt[:], pm[:, NSUB - 1:NSUB])
                nc.scalar.activation(
                    pm[:], pm[:], mybir.ActivationFunctionType.Sign,
                    bias=gt[:], scale=-1.0,
                    accum_out=Sf[:, rb:rb + 1])
                # taper the last groups so less recheck work is exposed
                # after the final row-block's drain
                if rb + 1 in group_starts:
                    gi = group_starts.index(rb + 1) - 1
                    r0, grp = group_starts[gi], rb + 1 - group_starts[gi]
                    Wt = emit_group_head(gi, r0, grp)
                    deferred = (gi, r0, grp, Wt)
                elif rb == n_rb - 1:
                    gi = len(group_starts) - 1
                    r0, grp = group_starts[gi], rb + 1 - group_starts[gi]
                    Wt = emit_group_head(gi, r0, grp)
                    emit_group_tail(gi, r0, grp, Wt)

            nc.sync.dma_start(out_d[:], Yg[:])

    if reuse_weights:
        _strip_redundant_ldweights(nc)
    nc.compile()
    return nc


def _strip_redundant_ldweights(nc):
    """Drop InstLdweights whose weights AP + tile_position match the previous
    load for that PE quadrant (the stationary operand is unchanged across the
    8 chunks of a row-block, and the s=0/1 groups live in disjoint quadrants).
    Only sync-free loads are dropped so semaphore chains stay intact."""
    dropped = 0
    for f in nc.m.functions:
        for blk in f.blocks:
            last_sig = {}
            kept = []
            for inst in blk.instructions:
                if type(inst).__name__ == "InstLdweights":
                    si = inst.sync_info
                    has_sync = si is not None and (
                        len(si.on_wait) > 0 or len(si.on_update) > 0)
                    tp = tuple(inst.tile_position or (0, 0))
                    w = inst.ins[0]
                    sig = (str(getattr(w, "offset", None)),
                           str(getattr(w, "ap", None)),
                           str(getattr(w, "tensor_name", None) or
                               getattr(w, "name", None)))
                    if not has_sync and last_sig.get(tp) == sig:
                        dropped += 1
                        continue
                    last_sig[tp] = sig
                kept.append(inst)
            blk.instructions[:] = kept
    assert dropped > 0, "expected to drop redundant ldweights"
    return dropped


def prep_inputs_v2(x, xb, y, nq=NQ, m=M):
    import ml_dtypes

    bf16 = ml_dtypes.bfloat16
    x = np.asarray(x, dtype=np.float32)
    xb = np.asarray(xb, dtype=np.float32)
    y = np.asarray(y, dtype=np.float32)
    n_rb = nq // RB
    n_chunk = m // CHUNK
    ncores = x.shape[0] // nq

    def bf(a):
        return a.astype(bf16).astype(np.float32)

    a = 2.0 * xb.T                       # [16, m]
    ah, al = bf(a), a - bf(a)
    b2 = -np.einsum("ij,ij->i", xb, xb)  # [m]
    b2h, b2l = bf(b2), b2 - bf(b2)
    R = np.zeros((K_SPL, m), np.float32)
    R[0:16] = ah
    R[16] = b2h
    R[17] = b2l
    R[18:34] = ah
    R[34:50] = al
    Rr = R.reshape(K_SPL, n_chunk, TPG, JT)  # u = 2*v + s on axis 2
    XB2 = np.zeros((128, n_chunk * 2, JT), np.float32)
    for s in range(2):
        XB2[64 * s:64 * s + K_SPL] = Rr[:, :, [s, 2 + s], :].reshape(
            K_SPL, n_chunk * 2, JT)
    xbp = np.ascontiguousarray(XB2.reshape(128, n_chunk * 2 * JT)).astype(bf16)

    xaug = np.empty((K_AUG, m), np.float32)
    xaug[:DIM] = 2.0 * xb.T
    xaug[DIM] = b2
    xw = np.ascontiguousarray(
        xaug.reshape(K_AUG, NSUB, WSUB).transpose(1, 2, 0).reshape(NSUB, WK))

    ytab = np.ascontiguousarray(y.reshape(m, 1))

    in_maps = []
    for c in range(ncores):
        xq = x[c * nq:(c + 1) * nq].T    # [16, nq]
        L = np.zeros((K_SPL, nq), np.float32)
        L[0:16] = bf(xq)
        L[16] = 1.0
        L[17] = 1.0
        L[18:34] = xq - bf(xq)
        L[34:50] = bf(xq)
        XQ2 = np.zeros((128, nq), np.float32)
        for s in range(2):
            XQ2[64 * s:64 * s + K_SPL] = L
        arr = np.ones((128, n_rb, K_AUG), np.float32)
        arr[:, :, :DIM] = x[c * nq:(c + 1) * nq].reshape(
            n_rb, RB, DIM).transpose(1, 0, 2)
        xqr = np.ascontiguousarray(arr.reshape(128, -1))
        in_maps.append({"xq4": XQ2.astype(bf16), "xbp": xbp, "xw": xw,
                        "xqr": xqr, "ytab": ytab})
    return in_maps


def unpack_output_v2(out_np, nq=NQ):
    return np.ascontiguousarray(out_np.T).reshape(nq)


_NC_CACHE = {}


def kernel(x, xb, y):
    import concourse.bass_utils as bass_utils

    if "v2" not in _NC_CACHE:
        _NC_CACHE["v2"] = build_nc_v2()
    nc = _NC_CACHE["v2"]
    in_maps = prep_inputs_v2(x, xb, y)
    res = bass_utils.run_bass_kernel_spmd(nc, in_maps,
                                          core_ids=list(range(NCORES)))
    outs = [unpack_output_v2(r["yout"]) for r in res.results]
    return np.concatenate(outs).astype(np.float32)


if __name__ == "__main__":
    rng = np.random.default_rng(0)
    x = rng.standard_normal((N, DIM), dtype=np.float32)
    xb = rng.standard_normal((M, DIM), dtype=np.float32)
    y = rng.random(M, dtype=np.float32)
    got = kernel(x, xb, y)
    d2 = (np.sum(x * x, 1)[:, None] + np.sum(xb * xb, 1)[None, :]
          - 2.0 * x @ xb.T)
    want = y[np.argmin(d2, axis=1)]
    err = np.abs(got - want)
    print("mismatches:", int((err > 0).sum()), "/", N)



# revision 2
# speedup vs baseline: 1.1272x; 1.1272x over previous
"""KNN retrieval kernel v4 for Trainium2 (8 NeuronCores, data-parallel over queries).

Per core: 2048 queries x 16384 refs, score ms[i,j] = 2<x_i,xb_j> - ||xb_j||^2.
Dual-port PSUM drain with 16 chunks of 1024 scores per row-block, 4 psum
tiles, roles [A,A,A,D]x4 so ScalarE (ACT) and VectorE (DVE) consume the psum
stream concurrently:
  - D-chunks {3,7,11,15}: DVE tensor_reduce (stride-32 windows of 32) ->
    exact fp32 window maxima.
  - A-chunks: ScalarE Identity-copy psum->SBUF fp16 with per-query bias
    (score - bias ~ 0 near the max => ~1e-3 absolute fp16 error); DVE
    tensor_tensor max trees (fp16 2x mode, 6-chunk batches) reduce each
    chunk to its 32 window maxima at ~0.5 cyc/score.
  - SM[128, 512] fp16 (biased domain); MAX8 + FIND_INDEX8 give top-2 windows.
  - Recheck: GPSIMD indirect-gathers the 2 windows' rows (32 refs x 18
    floats: aug vector + y), multiplies by the query aug vector and
    pre-folds 18->9; DVE re-dots exactly in fp32 and selects y[argmax] with
    an is_ge mask (y rides in the gather rows; no separate y gather).
Host: split-bf16 packing (hi*hi + b2 + lo*hi + hi*lo, K=50), window table,
and a least-squares ||q||-linear bias fit from a small probe.
"""

import sys

sys.path.insert(0, "/opt/trn_rl_repo")

import numpy as np

N, M, DIM = 16384, 16384, 16
NCORES = 8
NQ = N // NCORES
RB = 128
JT = 512
CHUNK = 1024
K_SPL = 50
WSUB = 32          # refs per window
NWIN = 512         # 16 chunks x 32 windows
K_AUG = 18         # 16 dims + b2 + y
WK = WSUB * K_AUG  # 576

D_CH = [3, 7, 11, 15]
A_B0 = [0, 1, 2, 4, 5, 6]
A_B1 = [8, 9, 10, 12, 13, 14]
SID2CHUNK = D_CH + A_B0 + A_B1


def build_nc_v4(nq=NQ, m=M):
    from contextlib import ExitStack

    import concourse.bacc as bacc
    import concourse.bass as bass
    import concourse.mybir as mybir
    import concourse.tile as tile
    from concourse.bass import IndirectOffsetOnAxis

    fp32 = mybir.dt.float32
    fp16 = mybir.dt.float16
    bf16 = mybir.dt.bfloat16
    u32 = mybir.dt.uint32

    n_rb = nq // RB
    n_chunk = m // CHUNK          # 16

    nc = bacc.Bacc("TRN2", target_bir_lowering=False, debug=False)

    xq_d = nc.dram_tensor("xq4", [128, nq], bf16, kind="ExternalInput")
    xb_d = nc.dram_tensor("xbp", [128, m // 2], bf16, kind="ExternalInput")
    xw_d = nc.dram_tensor("xw", [NWIN, WK], fp32, kind="ExternalInput")
    xqr_d = nc.dram_tensor("xqr", [128, n_rb * K_AUG], fp32,
                           kind="ExternalInput")
    nb_d = nc.dram_tensor("nbias", [128, n_rb], fp32, kind="ExternalInput")
    out_d = nc.dram_tensor("yout", [128, n_rb], fp32, kind="ExternalOutput")

    with tile.TileContext(nc) as tc:
        with ExitStack() as ctx:
            consts = ctx.enter_context(tc.tile_pool(name="consts", bufs=1))
            psum_pool = ctx.enter_context(
                tc.tile_pool(name="ps", bufs=4, space=bass.MemorySpace.PSUM))
            cppool = ctx.enter_context(tc.tile_pool(name="cp", bufs=3))
            tmppool = ctx.enter_context(tc.tile_pool(name="tmp", bufs=2))
            smpool = ctx.enter_context(tc.tile_pool(name="sm", bufs=3))
            gpool = ctx.enter_context(tc.tile_pool(name="g", bufs=2))

            xq4 = consts.tile([128, nq], bf16)
            xb = consts.tile([128, m // 2], bf16)
            xqr = consts.tile([128, n_rb * K_AUG], fp32)
            nbias = consts.tile([128, n_rb], fp32)
            Yg = consts.tile([128, n_rb], fp32)

            nc.sync.dma_start(xb[:, 0:2 * JT], xb_d[:, 0:2 * JT])
            nc.sync.dma_start(xq4[:, 0:RB], xq_d[:, 0:RB])
            nc.sync.dma_start(nbias[:], nb_d[:])
            nc.sync.dma_start(xq4[:, RB:], xq_d[:, RB:])
            for t in range(1, 8):
                nc.sync.dma_start(xb[:, t * 1024:(t + 1) * 1024],
                                  xb_d[:, t * 1024:(t + 1) * 1024])
            nc.sync.dma_start(xqr[:], xqr_d[:])

            def emit_mms(rb, c, ps):
                # chunk c (1024 cols): old-chunk t=c//2, v=c%2; s quadrants
                t, v = c // 2, c % 2
                for s in range(2):
                    mm = nc.tensor.matmul(
                        ps[:, s * JT:(s + 1) * JT],
                        xq4[64 * s:64 * s + K_SPL, rb * RB:(rb + 1) * RB],
                        xb[64 * s:64 * s + K_SPL,
                           (t * 2 + v) * JT:(t * 2 + v + 1) * JT],
                        start=True, stop=True,
                        tile_position=(64 * s, 0),
                    )
                    if c > 0:
                        mm.ins.ldweights = False

            def emit_tree(cp, SM, b):
                """Tree-max six fp16 1024-col chunk copies -> 6x32 window
                maxima into SM[:, 128 + b*192 : 128 + (b+1)*192]."""
                tA = tmppool.tile([128, 3072], fp16, name=f"tA{b}", tag="tA")
                tB = tmppool.tile([128, 1536], fp16, name=f"tB{b}", tag="tB")
                w = cp[:].rearrange("p (c n) -> p c n", n=512)   # [*, 12, 512]
                nc.vector.tensor_tensor(
                    tA[:].rearrange("p (c n) -> p c n", n=512),
                    w[:, 0::2, :], w[:, 1::2, :], op=mybir.AluOpType.max)
                u = tA[:].rearrange("p (c n) -> p c n", n=256)
                nc.vector.tensor_tensor(
                    tB[:].rearrange("p (c n) -> p c n", n=256),
                    u[:, 0::2, :], u[:, 1::2, :], op=mybir.AluOpType.max)
                u = tB[:].rearrange("p (c n) -> p c n", n=128)
                nc.vector.tensor_tensor(
                    tA[:, 0:768].rearrange("p (c n) -> p c n", n=128),
                    u[:, 0::2, :], u[:, 1::2, :], op=mybir.AluOpType.max)
                u = tA[:, 0:768].rearrange("p (c n) -> p c n", n=64)
                nc.vector.tensor_tensor(
                    tB[:, 0:384].rearrange("p (c n) -> p c n", n=64),
                    u[:, 0::2, :], u[:, 1::2, :], op=mybir.AluOpType.max)
                u = tB[:, 0:384].rearrange("p (c n) -> p c n", n=32)
                nc.vector.tensor_tensor(
                    SM[:, 128 + b * 192:128 + (b + 1) * 192]
                    .rearrange("p (c n) -> p c n", n=32),
                    u[:, 0::2, :], u[:, 1::2, :], op=mybir.AluOpType.max)

            def emit_recheck_tail(rb, Wt, Vt9):
                Dd = gpool.tile([128, 64], fp32, name=f"dd{rb}", tag="dd")
                mv = gpool.tile([128, 1], fp32, name=f"mv{rb}", tag="mv")
                Dsel = gpool.tile([128, 64], fp32, name=f"dsel{rb}",
                                  tag="dsel")
                nc.vector.tensor_reduce(
                    Dd[:], Vt9[:].rearrange("p (c k) -> p c k", k=9),
                    mybir.AxisListType.X, mybir.AluOpType.add)
                nc.vector.tensor_reduce(
                    mv[:], Dd[:], mybir.AxisListType.X, mybir.AluOpType.max)
                nc.vector.scalar_tensor_tensor(
                    Dsel[:], Dd[:], mv[:], Wt[:, 17:2 * WK:K_AUG],
                    mybir.AluOpType.is_ge, mybir.AluOpType.mult)
                nc.vector.tensor_reduce(
                    Yg[:, rb:rb + 1], Dsel[:], mybir.AxisListType.X,
                    mybir.AluOpType.max)

            deferred = None
            for rb in range(n_rb):
                SM = smpool.tile([128, NWIN], fp16, name=f"sm{rb}", tag="sm")
                SMdraw = smpool.tile([128, 128], fp32, name=f"smd{rb}",
                                     tag="smd")
                cps = [cppool.tile([128, 6 * CHUNK], fp16,
                                   name=f"cp{rb}_{b}", tag=f"cpb{b}")
                       for b in range(2)]
                a_idx = 0
                for c in range(n_chunk):
                    ps = psum_pool.tile([128, CHUNK], fp32)
                    emit_mms(rb, c, ps)
                    if c in D_CH:
                        di = D_CH.index(c)
                        nc.vector.tensor_reduce(
                            SMdraw[:, di * 32:(di + 1) * 32],
                            ps[:].rearrange("p (k w) -> p w k", w=32),
                            mybir.AxisListType.X, mybir.AluOpType.max)
                        if di == 1:
                            emit_tree(cps[0], SM, 0)
                            if deferred is not None:
                                emit_recheck_tail(*deferred)
                                deferred = None
                        elif di == 3:
                            emit_tree(cps[1], SM, 1)
                    else:
                        b, pos = a_idx // 6, a_idx % 6
                        nc.scalar.activation(
                            cps[b][:, pos * CHUNK:(pos + 1) * CHUNK], ps[:],
                            mybir.ActivationFunctionType.Identity,
                            bias=nbias[:, rb:rb + 1], scale=1.0)
                        a_idx += 1
                nc.vector.tensor_scalar_add(
                    SM[:, 0:128], SMdraw[:], nbias[:, rb:rb + 1])
                V8 = gpool.tile([128, 8], fp16, name=f"v8{rb}", tag="v8")
                I8 = gpool.tile([128, 8], u32, name=f"i8{rb}", tag="i8")
                nc.vector.max(V8[:], SM[:])
                nc.vector.max_index(I8[:], V8[:], SM[:])
                # recheck gathers + multiply + 18->9 fold on GPSIMD
                Wt = gpool.tile([128, 2 * WK], fp32, name=f"wt{rb}", tag="wt")
                Vt = gpool.tile([128, 2 * WK], fp32, name=f"vt{rb}", tag="vt")
                Vt9 = gpool.tile([128, 2 * WSUB * 9], fp32, name=f"v9{rb}",
                                 tag="v9")
                for g in range(2):
                    nc.gpsimd.indirect_dma_start(
                        Wt[:, g * WK:(g + 1) * WK], None, xw_d[:],
                        IndirectOffsetOnAxis(ap=I8[:, g:g + 1], axis=0))
                wv = Wt[:].rearrange("p (c k) -> p c k", k=K_AUG)
                xq_b = (xqr[:, rb * K_AUG:(rb + 1) * K_AUG]
                        .rearrange("p (c k) -> p c k", c=1)
                        .to_broadcast([128, 2 * WSUB, K_AUG]))
                nc.gpsimd.tensor_tensor(
                    Vt[:].rearrange("p (c k) -> p c k", k=K_AUG),
                    wv, xq_b, op=mybir.AluOpType.mult)
                vv = Vt[:].rearrange("p (c k) -> p c k", k=K_AUG)
                nc.gpsimd.tensor_tensor(
                    Vt9[:].rearrange("p (c k) -> p c k", k=9),
                    vv[:, :, 0:9], vv[:, :, 9:18], op=mybir.AluOpType.add)
                if rb == n_rb - 1:
                    emit_recheck_tail(rb, Wt, Vt9)
                else:
                    deferred = (rb, Wt, Vt9)

            nc.sync.dma_start(out_d[:], Yg[:])

    _strip_redundant_ldweights(nc)
    nc.compile()
    return nc


def _strip_redundant_ldweights(nc):
    dropped = 0
    for f in nc.m.functions:
        for blk in f.blocks:
            last_sig = {}
            kept = []
            for inst in blk.instructions:
                if type(inst).__name__ == "InstLdweights":
                    si = inst.sync_info
                    has_sync = si is not None and (
                        len(si.on_wait) > 0 or len(si.on_update) > 0)
                    tp = tuple(inst.tile_position or (0, 0))
                    w = inst.ins[0]
                    sig = (str(getattr(w, "offset", None)),
                           str(getattr(w, "ap", None)),
                           str(getattr(w, "tensor_name", None) or
                               getattr(w, "name", None)))
                    if not has_sync and last_sig.get(tp) == sig:
                        dropped += 1
                        continue
                    last_sig[tp] = sig
                kept.append(inst)
            blk.instructions[:] = kept
    assert dropped > 0, "expected to drop redundant ldweights"
    return dropped


def prep_inputs_v4(x, xb, y, nq=NQ, m=M):
    import ml_dtypes

    bf16 = ml_dtypes.bfloat16
    x = np.asarray(x, dtype=np.float32)
    xb = np.asarray(xb, dtype=np.float32)
    y = np.asarray(y, dtype=np.float32)
    n_rb = nq // RB
    ncores = x.shape[0] // nq

    def bf(a):
        return a.astype(bf16).astype(np.float32)

    a = 2.0 * xb.T
    ah, al = bf(a), a - bf(a)
    b2 = -np.einsum("ij,ij->i", xb, xb)
    b2h, b2l = bf(b2), b2 - bf(b2)
    R = np.zeros((K_SPL, m), np.float32)
    R[0:16] = ah
    R[16] = b2h
    R[17] = b2l
    R[18:34] = ah
    R[34:50] = al
    Rr = R.reshape(K_SPL, 8, 4, JT)
    XB2 = np.zeros((128, 16, JT), np.float32)
    for s in range(2):
        XB2[64 * s:64 * s + K_SPL] = Rr[:, :, [s, 2 + s], :].reshape(
            K_SPL, 16, JT)
    xbp = np.ascontiguousarray(XB2.reshape(128, m // 2)).astype(bf16)

    # window table: sid -> 32 refs x [2*r (16), -||r||^2, y]
    # window (chunk c, w) holds refs c*1024 + k*32 + w, k=0..31
    ref_id = np.empty((NWIN, WSUB), np.int64)
    for sid in range(NWIN):
        ch, w = SID2CHUNK[sid // 32], sid % 32
        ref_id[sid] = ch * CHUNK + np.arange(WSUB) * 32 + w
    ent = np.empty((NWIN, WSUB, K_AUG), np.float32)
    ent[:, :, :16] = 2.0 * xb[ref_id]
    ent[:, :, 16] = b2[ref_id]
    ent[:, :, 17] = y[ref_id]
    xw = np.ascontiguousarray(ent.reshape(NWIN, WK))

    qs = np.arange(0, x.shape[0], max(1, x.shape[0] // 512))
    sub = np.arange(0, m, 4)
    mp = (2.0 * x[qs] @ xb[sub].T + b2[sub][None, :]).max(1)
    nq_p = np.linalg.norm(x[qs], axis=1)
    ab, *_ = np.linalg.lstsq(
        np.stack([nq_p, np.ones_like(nq_p)], 1), mp, rcond=None)
    bias_all = ab[0] * np.linalg.norm(x, axis=1) + ab[1]

    in_maps = []
    for c in range(ncores):
        xq = x[c * nq:(c + 1) * nq].T
        L = np.zeros((K_SPL, nq), np.float32)
        L[0:16] = bf(xq)
        L[16] = 1.0
        L[17] = 1.0
        L[18:34] = xq - bf(xq)
        L[34:50] = bf(xq)
        XQ2 = np.zeros((128, nq), np.float32)
        for s in range(2):
            XQ2[64 * s:64 * s + K_SPL] = L
        arr = np.zeros((128, n_rb, K_AUG), np.float32)
        arr[:, :, :DIM] = x[c * nq:(c + 1) * nq].reshape(
            n_rb, RB, DIM).transpose(1, 0, 2)
        arr[:, :, 16] = 1.0
        xqr = np.ascontiguousarray(arr.reshape(128, -1))
        nb = np.ascontiguousarray(
            -bias_all[c * nq:(c + 1) * nq].reshape(n_rb, RB).T
        ).astype(np.float32)
        in_maps.append({"xq4": XQ2.astype(bf16), "xbp": xbp, "xw": xw,
                        "xqr": xqr, "nbias": nb})
    return in_maps


def unpack_output_v4(out_np, nq=NQ):
    return np.ascontiguousarray(out_np.T).reshape(nq)


build_nc_v3 = build_nc_v4
prep_inputs_v3 = prep_inputs_v4
unpack_output_v3 = unpack_output_v4

_NC_CACHE = {}


def kernel(x, xb, y):
    import concourse.bass_utils as bass_utils

    if "v4" not in _NC_CACHE:
        _NC_CACHE["v4"] = build_nc_v4()
    nc = _NC_CACHE["v4"]
    in_maps = prep_inputs_v4(x, xb, y)
    res = bass_utils.run_bass_kernel_spmd(nc, in_maps,
                                          core_ids=list(range(NCORES)))
    outs = [unpack_output_v4(r["yout"]) for r in res.results]
    return np.concatenate(outs).astype(np.float32)


if __name__ == "__main__":
    rng = np.random.default_rng(0)
    x = rng.standard_normal((N, DIM), dtype=np.float32)
    xb = rng.standard_normal((M, DIM), dtype=np.float32)
    y = rng.random(M, dtype=np.float32)
    got = kernel(x, xb, y)
    d2 = (np.sum(x * x, 1)[:, None] + np.sum(xb * xb, 1)[None, :]
          - 2.0 * x @ xb.T)
    want = y[np.argmin(d2, axis=1)]
    err = np.abs(got - want)
    print("mismatches:", int((err > 0).sum()), "/", N)


# revision 3
# speedup vs baseline: 1.1297x; 1.0022x over previous
"""KNN retrieval kernel v8 for Trainium2 (8 NeuronCores, data-parallel over queries).

Per core: 2048 queries x 16384 refs, score ms[i,j] = 2<x_i,xb_j> - ||xb_j||^2.
Dual-port PSUM drain with 16 chunks of 1024 scores per row-block, 4 psum
tiles, roles [A,A,A,D]x4 so ScalarE (ACT) and VectorE (DVE) consume the psum
stream concurrently:
  - D-chunks {3,7,11,15}: DVE tensor_reduce (stride-32 windows of 32) ->
    exact fp32 window maxima.
  - A-chunks: ScalarE Identity-copy psum->SBUF fp16 with per-query bias
    (score - bias ~ 0 near the max => ~1e-3 absolute fp16 error); DVE
    tensor_tensor max trees (fp16 2x mode, 6-chunk batches) reduce each
    chunk to its 32 window maxima at ~0.5 cyc/score.
  - SM[128, 512] fp16 (biased domain); MAX8 + FIND_INDEX8 give top-2 windows.
  - Recheck: GPSIMD indirect-gathers the 2 windows' rows (32 refs x 18
    floats: aug vector + y), multiplies by the query aug vector and
    pre-folds 18->9; DVE re-dots exactly in fp32 and selects y[argmax] with
    an is_ge mask (y rides in the gather rows; no separate y gather).
Host: split-bf16 packing (hi*hi + b2 + lo*hi + hi*lo, K=50), window table,
and a least-squares ||q||-linear bias fit from a small probe.
"""

import sys

sys.path.insert(0, "/opt/trn_rl_repo")

import numpy as np

N, M, DIM = 16384, 16384, 16
NCORES = 8
NQ = N // NCORES
RB = 128
JT = 512
CHUNK = 1024
K_SPL = 50
WSUB = 32          # refs per window
NWIN = 512         # 16 chunks x 32 windows
K_AUG = 18         # 16 dims + b2 + y
WK = WSUB * K_AUG  # 576

D_CH = [3, 7, 11]
A_B0 = [0, 1, 2, 4, 5, 6]
A_B1 = [8, 9, 10, 12, 13, 14, 15]
SID2CHUNK = D_CH + A_B0 + A_B1


def build_nc_v4(nq=NQ, m=M):
    from contextlib import ExitStack

    import concourse.bacc as bacc
    import concourse.bass as bass
    import concourse.mybir as mybir
    import concourse.tile as tile
    from concourse.bass import IndirectOffsetOnAxis

    fp32 = mybir.dt.float32
    fp16 = mybir.dt.float16
    bf16 = mybir.dt.bfloat16
    u32 = mybir.dt.uint32

    n_rb = nq // RB
    n_chunk = m // CHUNK          # 16

    nc = bacc.Bacc("TRN2", target_bir_lowering=False, debug=False)

    xq_d = nc.dram_tensor("xq4", [128, nq], bf16, kind="ExternalInput")
    xb_d = nc.dram_tensor("xbp", [128, m // 2], bf16, kind="ExternalInput")
    xw_d = nc.dram_tensor("xw", [NWIN, WK], fp32, kind="ExternalInput")
    xqr_d = nc.dram_tensor("xqr", [128, n_rb * K_AUG], fp32,
                           kind="ExternalInput")
    nb_d = nc.dram_tensor("nbias", [128, n_rb], fp32, kind="ExternalInput")
    out_d = nc.dram_tensor("yout", [128, n_rb], fp32, kind="ExternalOutput")

    with tile.TileContext(nc) as tc:
        with ExitStack() as ctx:
            consts = ctx.enter_context(tc.tile_pool(name="consts", bufs=1))
            psum_pool = ctx.enter_context(
                tc.tile_pool(name="ps", bufs=4, space=bass.MemorySpace.PSUM))
            cppool = ctx.enter_context(tc.tile_pool(name="cp", bufs=3))
            tmppool = ctx.enter_context(tc.tile_pool(name="tmp", bufs=2))
            smpool = ctx.enter_context(tc.tile_pool(name="sm", bufs=3))
            gpool = ctx.enter_context(tc.tile_pool(name="g", bufs=2))

            xq4 = consts.tile([128, nq], bf16)
            xb = consts.tile([128, m // 2], bf16)
            xqr = consts.tile([128, n_rb * K_AUG], fp32)
            nbias = consts.tile([128, n_rb], fp32)
            Yg = consts.tile([128, n_rb], fp32)

            nc.sync.dma_start(xb[:, 0:2 * JT], xb_d[:, 0:2 * JT])
            nc.sync.dma_start(xq4[:, 0:RB], xq_d[:, 0:RB])
            nc.sync.dma_start(nbias[:], nb_d[:])
            nc.sync.dma_start(xq4[:, RB:], xq_d[:, RB:])
            for t in range(1, 8):
                nc.sync.dma_start(xb[:, t * 1024:(t + 1) * 1024],
                                  xb_d[:, t * 1024:(t + 1) * 1024])
            nc.sync.dma_start(xqr[:], xqr_d[:])

            def emit_mms(rb, c, ps):
                # chunk c (1024 cols): old-chunk t=c//2, v=c%2; s quadrants
                t, v = c // 2, c % 2
                for s in range(2):
                    mm = nc.tensor.matmul(
                        ps[:, s * JT:(s + 1) * JT],
                        xq4[64 * s:64 * s + K_SPL, rb * RB:(rb + 1) * RB],
                        xb[64 * s:64 * s + K_SPL,
                           (t * 2 + v) * JT:(t * 2 + v + 1) * JT],
                        start=True, stop=True,
                        tile_position=(64 * s, 0),
                    )
                    if c > 0:
                        mm.ins.ldweights = False

            b_off = [96, 288]
            b_nch = [6, 7]

            def emit_tree(cp, SM, b):
                """fp16 TT-max tree: nch 1024-col copies -> nch*32 windows."""
                nch = b_nch[b]
                tA = tmppool.tile([128, nch * 512], fp16, name=f"tA{b}",
                                  tag=f"tA{b}")
                tB = tmppool.tile([128, nch * 256], fp16, name=f"tB{b}",
                                  tag=f"tB{b}")
                w = cp[:].rearrange("p (c n) -> p c n", n=512)
                nc.vector.tensor_tensor(
                    tA[:].rearrange("p (c n) -> p c n", n=512),
                    w[:, 0::2, :], w[:, 1::2, :], op=mybir.AluOpType.max)
                u = tA[:].rearrange("p (c n) -> p c n", n=256)
                nc.vector.tensor_tensor(
                    tB[:].rearrange("p (c n) -> p c n", n=256),
                    u[:, 0::2, :], u[:, 1::2, :], op=mybir.AluOpType.max)
                u = tB[:].rearrange("p (c n) -> p c n", n=128)
                nc.vector.tensor_tensor(
                    tA[:, 0:nch * 128].rearrange("p (c n) -> p c n", n=128),
                    u[:, 0::2, :], u[:, 1::2, :], op=mybir.AluOpType.max)
                u = tA[:, 0:nch * 128].rearrange("p (c n) -> p c n", n=64)
                nc.vector.tensor_tensor(
                    tB[:, 0:nch * 64].rearrange("p (c n) -> p c n", n=64),
                    u[:, 0::2, :], u[:, 1::2, :], op=mybir.AluOpType.max)
                u = tB[:, 0:nch * 64].rearrange("p (c n) -> p c n", n=32)
                nc.vector.tensor_tensor(
                    SM[:, b_off[b]:b_off[b] + nch * 32]
                    .rearrange("p (c n) -> p c n", n=32),
                    u[:, 0::2, :], u[:, 1::2, :], op=mybir.AluOpType.max)

            def emit_recheck_tail(rb, Wt, Vt9):
                Dd = gpool.tile([128, 64], fp32, name=f"dd{rb}", tag="dd")
                mv = gpool.tile([128, 1], fp32, name=f"mv{rb}", tag="mv")
                Dsel = gpool.tile([128, 64], fp32, name=f"dsel{rb}",
                                  tag="dsel")
                nc.vector.tensor_reduce(
                    Dd[:], Vt9[:].rearrange("p (c k) -> p c k", k=9),
                    mybir.AxisListType.X, mybir.AluOpType.add)
                nc.vector.tensor_reduce(
                    mv[:], Dd[:], mybir.AxisListType.X, mybir.AluOpType.max)
                nc.vector.scalar_tensor_tensor(
                    Dsel[:], Dd[:], mv[:], Wt[:, 17:2 * WK:K_AUG],
                    mybir.AluOpType.is_ge, mybir.AluOpType.mult)
                nc.vector.tensor_reduce(
                    Yg[:, rb:rb + 1], Dsel[:], mybir.AxisListType.X,
                    mybir.AluOpType.max)

            deferred = None
            for rb in range(n_rb):
                SM = smpool.tile([128, NWIN], fp16, name=f"sm{rb}", tag="sm")
                SMdraw = smpool.tile([128, 128], fp32, name=f"smd{rb}",
                                     tag="smd")
                cps = [cppool.tile([128, b_nch[b] * CHUNK], fp16,
                                   name=f"cp{rb}_{b}", tag=f"cpb{b}")
                       for b in range(2)]
                a_idx = 0
                for c in range(n_chunk):
                    ps = psum_pool.tile([128, CHUNK], fp32)
                    emit_mms(rb, c, ps)
                    if c in D_CH:
                        di = D_CH.index(c)
                        nc.vector.tensor_reduce(
                            SMdraw[:, di * 32:(di + 1) * 32],
                            ps[:].rearrange("p (k w) -> p w k", w=32),
                            mybir.AxisListType.X, mybir.AluOpType.max)
                        if c == 7:
                            emit_tree(cps[0], SM, 0)
                            if deferred is not None:
                                emit_recheck_tail(*deferred)
                                deferred = None
                    else:
                        b, pos = (0, a_idx) if a_idx < 6 else (1, a_idx - 6)
                        nc.scalar.activation(
                            cps[b][:, pos * CHUNK:(pos + 1) * CHUNK], ps[:],
                            mybir.ActivationFunctionType.Identity,
                            bias=nbias[:, rb:rb + 1], scale=1.0)
                        a_idx += 1
                emit_tree(cps[1], SM, 1)
                nc.vector.tensor_scalar_add(
                    SM[:, 0:96], SMdraw[:, 0:96], nbias[:, rb:rb + 1])
                V8 = gpool.tile([128, 8], fp16, name=f"v8{rb}", tag="v8")
                I8 = gpool.tile([128, 8], u32, name=f"i8{rb}", tag="i8")
                nc.vector.max(V8[:], SM[:])
                nc.vector.max_index(I8[:], V8[:], SM[:])
                # recheck gathers + multiply + 18->9 fold on GPSIMD
                Wt = gpool.tile([128, 2 * WK], fp32, name=f"wt{rb}", tag="wt")
                Vt = gpool.tile([128, 2 * WK], fp32, name=f"vt{rb}", tag="vt")
                Vt9 = gpool.tile([128, 2 * WSUB * 9], fp32, name=f"v9{rb}",
                                 tag="v9")
                for g in range(2):
                    nc.gpsimd.indirect_dma_start(
                        Wt[:, g * WK:(g + 1) * WK], None, xw_d[:],
                        IndirectOffsetOnAxis(ap=I8[:, g:g + 1], axis=0))
                wv = Wt[:].rearrange("p (c k) -> p c k", k=K_AUG)
                xq_b = (xqr[:, rb * K_AUG:(rb + 1) * K_AUG]
                        .rearrange("p (c k) -> p c k", c=1)
                        .to_broadcast([128, 2 * WSUB, K_AUG]))
                nc.gpsimd.tensor_tensor(
                    Vt[:].rearrange("p (c k) -> p c k", k=K_AUG),
                    wv, xq_b, op=mybir.AluOpType.mult)
                vv = Vt[:].rearrange("p (c k) -> p c k", k=K_AUG)
                nc.gpsimd.tensor_tensor(
                    Vt9[:].rearrange("p (c k) -> p c k", k=9),
                    vv[:, :, 0:9], vv[:, :, 9:18], op=mybir.AluOpType.add)
                if rb == n_rb - 1:
                    emit_recheck_tail(rb, Wt, Vt9)
                else:
                    deferred = (rb, Wt, Vt9)

            nc.sync.dma_start(out_d[:], Yg[:])

    _strip_redundant_ldweights(nc)
    nc.compile()
    return nc


def _strip_redundant_ldweights(nc):
    dropped = 0
    for f in nc.m.functions:
        for blk in f.blocks:
            last_sig = {}
            kept = []
            for inst in blk.instructions:
                if type(inst).__name__ == "InstLdweights":
                    si = inst.sync_info
                    has_sync = si is not None and (
                        len(si.on_wait) > 0 or len(si.on_update) > 0)
                    tp = tuple(inst.tile_position or (0, 0))
                    w = inst.ins[0]
                    sig = (str(getattr(w, "offset", None)),
                           str(getattr(w, "ap", None)),
                           str(getattr(w, "tensor_name", None) or
                               getattr(w, "name", None)))
                    if not has_sync and last_sig.get(tp) == sig:
                        dropped += 1
                        continue
                    last_sig[tp] = sig
                kept.append(inst)
            blk.instructions[:] = kept
    assert dropped > 0, "expected to drop redundant ldweights"
    return dropped


def prep_inputs_v4(x, xb, y, nq=NQ, m=M):
    import ml_dtypes

    bf16 = ml_dtypes.bfloat16
    x = np.asarray(x, dtype=np.float32)
    xb = np.asarray(xb, dtype=np.float32)
    y = np.asarray(y, dtype=np.float32)
    n_rb = nq // RB
    ncores = x.shape[0] // nq

    def bf(a):
        return a.astype(bf16).astype(np.float32)

    a = 2.0 * xb.T
    ah, al = bf(a), a - bf(a)
    b2 = -np.einsum("ij,ij->i", xb, xb)
    b2h, b2l = bf(b2), b2 - bf(b2)
    R = np.zeros((K_SPL, m), np.float32)
    R[0:16] = ah
    R[16] = b2h
    R[17] = b2l
    R[18:34] = ah
    R[34:50] = al
    Rr = R.reshape(K_SPL, 8, 4, JT)
    XB2 = np.zeros((128, 16, JT), np.float32)
    for s in range(2):
        XB2[64 * s:64 * s + K_SPL] = Rr[:, :, [s, 2 + s], :].reshape(
            K_SPL, 16, JT)
    xbp = np.ascontiguousarray(XB2.reshape(128, m // 2)).astype(bf16)

    # window table: sid -> 32 refs x [2*r (16), -||r||^2, y]
    # window (chunk c, w) holds refs c*1024 + k*32 + w, k=0..31
    ref_id = np.empty((NWIN, WSUB), np.int64)
    for sid in range(NWIN):
        ch, w = SID2CHUNK[sid // 32], sid % 32
        ref_id[sid] = ch * CHUNK + np.arange(WSUB) * 32 + w
    ent = np.empty((NWIN, WSUB, K_AUG), np.float32)
    ent[:, :, :16] = 2.0 * xb[ref_id]
    ent[:, :, 16] = b2[ref_id]
    ent[:, :, 17] = y[ref_id]
    xw = np.ascontiguousarray(ent.reshape(NWIN, WK))

    qs = np.arange(0, x.shape[0], max(1, x.shape[0] // 512))
    sub = np.arange(0, m, 4)
    mp = (2.0 * x[qs] @ xb[sub].T + b2[sub][None, :]).max(1)
    nq_p = np.linalg.norm(x[qs], axis=1)
    ab, *_ = np.linalg.lstsq(
        np.stack([nq_p, np.ones_like(nq_p)], 1), mp, rcond=None)
    bias_all = ab[0] * np.linalg.norm(x, axis=1) + ab[1]

    in_maps = []
    for c in range(ncores):
        xq = x[c * nq:(c + 1) * nq].T
        L = np.zeros((K_SPL, nq), np.float32)
        L[0:16] = bf(xq)
        L[16] = 1.0
        L[17] = 1.0
        L[18:34] = xq - bf(xq)
        L[34:50] = bf(xq)
        XQ2 = np.zeros((128, nq), np.float32)
        for s in range(2):
            XQ2[64 * s:64 * s + K_SPL] = L
        arr = np.zeros((128, n_rb, K_AUG), np.float32)
        arr[:, :, :DIM] = x[c * nq:(c + 1) * nq].reshape(
            n_rb, RB, DIM).transpose(1, 0, 2)
        arr[:, :, 16] = 1.0
        xqr = np.ascontiguousarray(arr.reshape(128, -1))
        nb = np.ascontiguousarray(
            -bias_all[c * nq:(c + 1) * nq].reshape(n_rb, RB).T
        ).astype(np.float32)
        in_maps.append({"xq4": XQ2.astype(bf16), "xbp": xbp, "xw": xw,
                        "xqr": xqr, "nbias": nb})
    return in_maps


def unpack_output_v4(out_np, nq=NQ):
    return np.ascontiguousarray(out_np.T).reshape(nq)


build_nc_v3 = build_nc_v4
prep_inputs_v3 = prep_inputs_v4
unpack_output_v3 = unpack_output_v4

_NC_CACHE = {}


def kernel(x, xb, y):
    import concourse.bass_utils as bass_utils

    if "v4" not in _NC_CACHE:
        _NC_CACHE["v4"] = build_nc_v4()
    nc = _NC_CACHE["v4"]
    in_maps = prep_inputs_v4(x, xb, y)
    res = bass_utils.run_bass_kernel_spmd(nc, in_maps,
                                          core_ids=list(range(NCORES)))
    outs = [unpack_output_v4(r["yout"]) for r in res.results]
    return np.concatenate(outs).astype(np.float32)


if __name__ == "__main__":
    rng = np.random.default_rng(0)
    x = rng.standard_normal((N, DIM), dtype=np.float32)
    xb = rng.standard_normal((M, DIM), dtype=np.float32)
    y = rng.random(M, dtype=np.float32)
    got = kernel(x, xb, y)
    d2 = (np.sum(x * x, 1)[:, None] + np.sum(xb * xb, 1)[None, :]
          - 2.0 * x @ xb.T)
    want = y[np.argmin(d2, axis=1)]
    err = np.abs(got - want)
    print("mismatches:", int((err > 0).sum()), "/", N)
